# revision 49
# baseline (speedup 1.0000x reference)
"""Tensor-parallel causal multi-head attention (RoPE) for 8 Trainium2 NeuronCores.

Problem: B=1, S=2048, D=4096, H=32 heads, head_dim=128, causal, RoPE,
out-projection with bias.  Reference: y = softmax(mask(QK^T/sqrt(hd))) V Wo^T + bo
with Q/K/V = X @ W{q,k,v}^T (nn.Linear convention) and RoPE applied to Q, K.

Sharding: tensor-parallel across heads (4 heads / core) for QKV + attention;
AllToAll re-shards to sequence (256 rows / core) for the out-projection.
Each core returns its 256-row slice of the final output; host concatenates.

v5 vs v2 (783us -> ~696us on HW; PE is GPIO-power-throttled to ~1.95GHz
for ~85% of every run, so the N=512 bf16 matmul floor is 263ns):
- scores/exp/PV restricted to the causal region at k-tile granularity
  (diagonal 512-block tiles shrink to 512/384/256/128 columns, exactly
  the per-k-tile causal limit).
- exp fused over two k-tiles per ACTIVATE ([128,1024] fp32 PSUM pair):
  the 352-cycle fixed cost was 40% of each [128,512] exp.
- per-k-tile PE colsum matmuls (~42us of PE streaming) replaced by a
  bf16 tree-sum on the DVE plus ONE all-ones [128,128] matmul per
  q-chunk that does the cross-partition reduce AND the partition
  broadcast in one pass; normalization is deferred into the next
  q-chunk's instruction stream so the PE never waits on the tree.
- V projection computed pre-transposed (stationary = X k-tile, moving =
  resident Wv^T): the 64 PE transposes of v2 vanish.
- resident cos/sin (loaded first, via SWDGE) with the 1/sqrt(hd) scale
  folded into Wq host-side; RoPE rot DMAs ride the otherwise-idle
  scalar HWDGE queue (behind bulk prefetch they stalled PSUM recycling
  ~30us; on gpsimd they deadlock behind the launch-skew barrier).
- out-projection weight slabs prefetched two od-slices ahead in the
  tail; oproj(1)+oproj(2) (~68us of PE work) emitted after the last
  AllToAll's trigger covers its ~15-45us latency.
Measured end-to-end rel_l2 ~5.9e-3 vs the 2e-2 gate.
"""

import sys
import numpy as np

for _p in ("/opt/trn_rl_repo",):
    if _p not in sys.path:
        sys.path.insert(0, _p)

B, S, D, H = 1, 2048, 4096, 32
HD = 128          # head dim
NC = 8            # cores
HPC = H // NC     # heads per core = 4
MPC = 3 * HPC     # projection m-tiles per core (Q0..3, K0..3, V0..3) = 12
SQ = S // NC      # seq rows per core after AllToAll = 256
KT = D // 128     # contraction tiles = 32

_cache = {}


def _build_program():
    import concourse.bass as bass
    import concourse.mybir as mybir
    import concourse.tile as tile
    from concourse import bacc
    from contextlib import ExitStack

    F32 = mybir.dt.float32
    F32R = mybir.dt.float32r
    BF16 = mybir.dt.bfloat16
    PF32 = mybir.dt.float32
    AF = mybir.ActivationFunctionType

    nc = bacc.Bacc("TRN2", target_bir_lowering=False, debug=False, num_devices=NC)

    XT = nc.dram_tensor("XT", [D, S], BF16, kind="ExternalInput")
    W4 = nc.dram_tensor("W4", [8, 128, KT, 128], BF16, kind="ExternalInput")
    WVT = nc.dram_tensor("WVT", [128, KT, 512], BF16, kind="ExternalInput")
    COS = nc.dram_tensor("COS", [128, S], F32, kind="ExternalInput")
    SIN = nc.dram_tensor("SIN", [128, S], F32, kind="ExternalInput")
    MASK1 = nc.dram_tensor("MASK1", [128, 128], BF16, kind="ExternalInput")
    ONESM = nc.dram_tensor("ONESM", [128, 128], BF16, kind="ExternalInput")
    WOT = nc.dram_tensor("WOT", [HPC, 8, 8, 128, 512], BF16, kind="ExternalInput")
    BOB = nc.dram_tensor("BOB", [128, D], F32, kind="ExternalInput")
    Y = nc.dram_tensor("Y", [SQ, D], F32, kind="ExternalOutput")

    with tile.TileContext(nc) as tc, ExitStack() as top:
        dram = top.enter_context(tc.tile_pool(name="dram", bufs=1, space="DRAM"))
        a2a_in = [dram.tile([NC, HD, SQ], BF16, name=f"a2ai{h}") for h in range(HPC)]
        a2a_out = [dram.tile([NC, HD, SQ], BF16, name=f"a2ao{h}") for h in range(HPC)]
        sync_in = dram.tile([NC, 1, 8], BF16, name="syncin")
        sync_out = dram.tile([NC, 1, 8], BF16, name="syncout")

        # constants via the software-DGE queue (gpsimd): keeps both HWDGE
        # queues clear for the startup X/W stream.  cos/sin go FIRST -- the
        # very first RoPE evacuate (~18us in) consumes them; in v2 they sat
        # behind 3MB of X pieces on the scalar HWDGE queue and the first
        # evacuate chain stalled the PE for 19us.
        # Persistent Q^T/K^T ([hd, s], RoPE'd) and V ([s%128, s//128, 4*hd]:
        # the V projection is computed directly transposed — stationary = X
        # k-tile, moving = Wv^T — so the 64 PE transposes of v2-v4 vanish).
        res = top.enter_context(tc.tile_pool(name="res", bufs=1))
        qk_res = [res.tile([128, S], BF16, name=f"qk{m}") for m in range(8)]
        v_res2 = res.tile([128, S // 128, 512], BF16, name="v2")

        XT_t = XT.ap().rearrange("(k p) s -> p k s", p=128)

        # ---- Phase 1: QKV projections + RoPE (Q,K) + transpose (V) ----
        # s-chunk outer (4 x 512 cols), m-tile inner; the full K=4096
        # contraction accumulates in one PSUM bank (32 matmuls).  X streams
        # once (double-buffered 4MB chunks), W streams once per chunk.
        with nc.named_scope("proj"), ExitStack() as ph:
            xt_pool = ph.enter_context(tc.tile_pool(name="xt", bufs=2))
            w_pool = ph.enter_context(tc.tile_pool(name="w", bufs=3))
            cs_pool = ph.enter_context(tc.tile_pool(name="cs", bufs=1))
            rtmp = ph.enter_context(tc.tile_pool(name="rtmp", bufs=2))
            # resident cos/sin, scoped to the projection phase (freed for
            # the attention pools); software-DGE so they land before the
            # first evacuate without fighting the X/W startup stream.
            cos_sb = cs_pool.tile([128, S], F32, name="cos")
            sin_sb = cs_pool.tile([128, S], F32, name="sin")

            # staged per s-chunk: only chunk 0's slice competes with the
            # startup X/W stream (the full 2MB stole ~6us of the HBM
            # bandwidth that the first chain is starved for).  Later
            # slices are emitted ahead of the gpsimd barrier dwell.
            def load_cs(a):
                sl = slice(a * 512, (a + 1) * 512)
                nc.gpsimd.dma_start(cos_sb[:, sl], COS.ap()[:, sl])
                nc.gpsimd.dma_start(sin_sb[:, sl], SIN.ap()[:, sl])

            load_cs(0)
            full_pool = ph.enter_context(tc.tile_pool(name="full", bufs=2))
            ppsum = ph.enter_context(tc.tile_pool(name="ppsum", bufs=4, space="PSUM"))
            # Wv^T stays resident (4MB): every s-tile's chain contracts all
            # of it, so streaming it per chunk would cost 16MB.
            wvt_sb = cs_pool.tile([128, KT, 512], BF16, name="wvt")

            def evacuate_qk(m, a, ps):
                s0 = a * 512
                sl = slice(s0, s0 + 512)
                # 1/sqrt(hd) is folded into the Wq slabs host-side, so Q
                # and K share one resident cos/sin pair.
                full_sb = full_pool.tile([128, 512], F32R, name="full")
                # ACT does the PSUM evacuation copy: it idles all through
                # proj, while the DVE's copy backlog at the proj->attn
                # boundary was stalling attention heads 0-1 (~14us).
                nc.scalar.copy(full_sb[:], ps[:])
                # rotate-half as a partition swap via SBUF->SBUF DMA
                # (sign is folded into SIN host-side) — keeps the
                # rotation off the PE (was a 548ns fp32 matmul each).
                # Both halves ride the scalar HWDGE queue, which carries
                # no bulk traffic after startup: behind 512KB prefetch
                # pieces the whole evacuate chain (and PSUM recycling)
                # stalls ~30us; on gpsimd it deadlocks ~40us behind the
                # launch-skew barrier collective.
                rot_sb = rtmp.tile([128, 512], F32R, name="rot")
                nc.scalar.dma_start(rot_sb[0:64, :], full_sb[64:128, :])
                nc.scalar.dma_start(rot_sb[64:128, :], full_sb[0:64, :])
                # last chunk's LATE slots (consumed only by attention heads
                # 2-3, ~60us later) run their RoPE arithmetic on gpsimd:
                # their DVE tail otherwise overlaps attention heads 0-1 and
                # the queued-behind masks/tree stall PV (~15us of PE
                # bubbles).  Head 0/1 inputs (Q0/K0/Q1/K1) stay on the DVE
                # so the attention start is not delayed by Pool's 2x-slower
                # elementwise path.
                # (Q0/K0/Q1/K1 stay on DVE — heads 0-1 need them promptly;
                # pushing Q1/K1 to Pool as well measured 5us slower.)
                eng = nc.gpsimd if (a == 3 and m in (2, 6, 3, 7)) else nc.vector
                c1 = rtmp.tile([128, 512], F32, name="c1")
                eng.tensor_mul(c1[:], full_sb[:], cos_sb[:, sl])
                r1 = rtmp.tile([128, 512], F32, name="r1")
                eng.tensor_mul(r1[:], rot_sb[:], sin_sb[:, sl])
                eng.tensor_add(qk_res[m][:, sl], c1[:], r1[:])

            def load_w(m):
                w_sb = w_pool.tile([128, KT, 128], BF16, name="w")
                nc.sync.dma_start(w_sb[:], W4.ap()[m])
                return w_sb

            # Startup interleave: first W slab goes out first (the first
            # matmul chain needs it plus only the first X piece), X pieces
            # pace the first chain, later W slabs slot between pieces.
            kq = KT // 8
            xt_cur = xt_pool.tile([128, KT, 512], BF16, name="xt")

            def xt0_piece(s, eng):
                eng.dma_start(xt_cur[:, s * kq : (s + 1) * kq, :],
                              XT_t[:, s * kq : (s + 1) * kq, 0:512])

            # startup: chunk-0 pieces split across both HWDGE queues so the
            # 4MB load lands in ~half the serial time; W slabs interleave
            # on the sync queue.  Slab 0 goes in quarters so the first
            # matmul only waits on a 256KB load.
            # strict alternation: even pieces on scalar, odd pieces on
            # sync woven between the w0 quarters, W slabs for slots 1-2
            # after the last piece — both queues then deliver chain 0's
            # inputs in consumption order.
            w0_sb = w_pool.tile([128, KT, 128], BF16, name="w")
            nc.sync.dma_start(w0_sb[:, 0:8, :], W4.ap()[0, :, 0:8, :])
            xt0_piece(0, nc.scalar)
            xt0_piece(1, nc.sync)
            xt0_piece(2, nc.scalar)
            nc.sync.dma_start(w0_sb[:, 8:16, :], W4.ap()[0, :, 8:16, :])
            xt0_piece(3, nc.sync)
            xt0_piece(4, nc.scalar)
            nc.sync.dma_start(w0_sb[:, 16:24, :], W4.ap()[0, :, 16:24, :])
            w_pre = [w0_sb]
            xt0_piece(5, nc.sync)
            xt0_piece(6, nc.scalar)
            nc.sync.dma_start(w0_sb[:, 24:32, :], W4.ap()[0, :, 24:32, :])
            xt0_piece(7, nc.sync)
            w_pre.append(load_w(4))
            w_pre.append(load_w(1))
            # slot 3's K1 slab must precede the 4MB Wv^T stream on sync or
            # the PE stalls 13us at ~55us waiting for it.
            w_pre.append(load_w(5))
            # resident Wv^T streams in behind the chunk-0 startup set; the
            # first V s-tile runs at slot 4 (~35us in).  Pieces interleave
            # with the in-loop W slab loads (below) so slot 2/3's K slabs
            # aren't queued behind all 4MB.
            def load_wvt(p):
                nc.sync.dma_start(wvt_sb[:, p * 8 : (p + 1) * 8, :],
                                  WVT.ap()[:, p * 8 : (p + 1) * 8, :])

            load_wvt(0)
            # chunk 0 defers its V s-tiles until Wv^T has landed.
            slots0 = [("qk", 0), ("qk", 4), ("qk", 1), ("qk", 5), ("v", 0),
                      ("v", 1), ("qk", 2), ("qk", 6), ("v", 2), ("qk", 3),
                      ("qk", 7), ("v", 3)]
            slotsN = [("qk", 0), ("qk", 4), ("v", 0), ("qk", 1), ("qk", 5),
                      ("v", 1), ("qk", 2), ("qk", 6), ("v", 2), ("qk", 3),
                      ("qk", 7), ("v", 3)]
            xt_next = None
            for a in range(4):
                for mi, (kind, m) in enumerate(slots0 if a == 0 else slotsN):
                    if a < 3:
                        # spread next-chunk prefetch: one 512KB piece per m,
                        # on the sync queue behind the W slabs (which run
                        # 2-3 slots ahead) — the scalar queue stays clear
                        # for the latency-critical RoPE rot DMAs.
                        if mi == 2:
                            xt_next = xt_pool.tile([128, KT, 512], BF16,
                                                   name="xt")
                        if 2 <= mi < 10:
                            s = mi - 2
                            nc.sync.dma_start(
                                xt_next[:, s * kq : (s + 1) * kq, :],
                                XT_t[:, s * kq : (s + 1) * kq,
                                     (a + 1) * 512 : (a + 2) * 512],
                            )
                    ps = ppsum.tile([128, 512], PF32, name="proj")
                    if a == 0 and mi in (0, 1, 2):
                        load_wvt(mi + 1)
                    if a < 3 and mi == 0:
                        load_cs(a + 1)
                    if kind == "qk":
                        w_sb = (w_pre[{0: 0, 4: 1, 1: 2, 5: 3}[m]]
                                if (a == 0 and mi < 4) else load_w(m))
                        for kt in range(KT):
                            nc.tensor.matmul(ps[:], w_sb[:, kt, :],
                                             xt_cur[:, kt, :],
                                             start=(kt == 0),
                                             stop=(kt == KT - 1))
                        evacuate_qk(m, a, ps)
                    else:
                        # V s-tile, computed transposed: stationary = X
                        # k-tile [k, s], moving = Wv^T [k, 4*hd]
                        t = 4 * a + m
                        jj = slice(m * 128, (m + 1) * 128)
                        for kt in range(KT):
                            nc.tensor.matmul(ps[:], xt_cur[:, kt, jj],
                                             wvt_sb[:, kt, :],
                                             start=(kt == 0),
                                             stop=(kt == KT - 1))
                        if a == 3:
                            nc.scalar.copy(v_res2[:, t, :], ps[:])
                        else:
                            nc.vector.tensor_copy(v_res2[:, t, :], ps[:])
                    if a == 0 and mi == 0:
                        # tiny collective doubling as a cross-core barrier:
                        # absorbs SPMD launch skew here, where the PE still
                        # has ~400us of queued work, so the first real
                        # AllToAll is not a skew sink.
                        zz = full_pool.tile([1, NC * 8], BF16, name="zz")
                        nc.any.memset(zz[:], 0.0)
                        nc.sync.dma_start(sync_in.rearrange("i p q -> p (i q)"),
                                          zz[:])
                        nc.gpsimd.collective_compute(
                            "AllToAll",
                            mybir.AluOpType.bypass,
                            replica_groups=[list(range(NC))],
                            ins=[sync_in.opt()],
                            outs=[sync_out.opt()],
                        )
                xt_cur = xt_next

        # ---- Phase 2+3: attention interleaved with out-projection ----
        # v4: scores/exp/PV restricted to the causal region at k-tile
        # granularity (the diagonal 512-block's tiles shrink to
        # 512/384/256/128 columns), exp fused over two k-tiles per
        # ACTIVATE (the 352-cycle fixed cost was 40% of each exp), and the
        # per-tile PE colsum matmuls (42us of streaming) replaced by a
        # bf16 tree-sum on the DVE plus ONE all-ones matmul per q-chunk
        # that does the cross-partition reduce and the partition broadcast
        # in the same pass.
        with nc.named_scope("attn"), ExitStack() as ph:
            es_pool = ph.enter_context(tc.tile_pool(name="es", bufs=2))
            tr_pool = ph.enter_context(tc.tile_pool(name="tr", bufs=2))
            den_pool = ph.enter_context(tc.tile_pool(name="den", bufs=2))
            on_pool = ph.enter_context(tc.tile_pool(name="on", bufs=4))
            of_pool = ph.enter_context(tc.tile_pool(name="of", bufs=2))
            wo_pool = ph.enter_context(tc.tile_pool(name="wo", bufs=8))
            yac_pool = ph.enter_context(tc.tile_pool(name="yac", bufs=1))
            yev_pool = ph.enter_context(tc.tile_pool(name="yev", bufs=4))
            msk_pool = ph.enter_context(tc.tile_pool(name="msk", bufs=1))
            spair = ph.enter_context(tc.tile_pool(name="spair", bufs=2, space="PSUM"))
            opsum = ph.enter_context(tc.tile_pool(name="opsum", bufs=2, space="PSUM"))
            aux_ps = ph.enter_context(tc.tile_pool(name="auxps", bufs=2, space="PSUM"))

            mask_sb = msk_pool.tile([128, 128], BF16, name="mask1")
            nc.sync.dma_start(mask_sb[:], MASK1.ap())
            onesm_sb = msk_pool.tile([128, 128], BF16, name="onesm")
            nc.sync.dma_start(onesm_sb[:], ONESM.ap())
            bias_sb = msk_pool.tile([128, D], F32, name="bias")
            nc.sync.dma_start(bias_sb[:], BOB.ap())
            yac = [yac_pool.tile([128, D], F32, name=f"yac{q2}") for q2 in range(2)]
            of_tiles = {}
            pending_norm = []

            def flush_norm():
                while pending_norm:
                    pending_norm.pop(0)()

            def mk_norm(h, qr, den_ap, ops):
                # one matmul against all-ones [128,128] = colsum over the
                # 128 k-partitions AND broadcast of the result to every
                # output partition; reciprocal+scale on DVE.
                def norm():
                    bps = aux_ps.tile([128, 512], PF32, name="aux")
                    nc.tensor.matmul(bps[:], onesm_sb[:], den_ap,
                                     start=True, stop=True)
                    rec_sb = on_pool.tile([128, 512], F32, name="rec")
                    nc.vector.reciprocal_approx_fast(out=rec_sb[:], in_=bps[:])
                    otn = on_pool.tile([128, 512], BF16, name="otn")
                    nc.vector.tensor_mul(otn[:], ops[:], rec_sb[:])
                    for half in range(2):
                        nc.sync.dma_start(
                            a2a_in[h][2 * qr + half, :, :],
                            otn[:, half * 256 : (half + 1) * 256],
                        )
                return norm

            def attn_qr(h, qr):
                nk = 4 * (qr + 1)
                q0 = qr * 512
                base = 4 * qr * 512  # es offset of the diagonal region
                # (kt, width, q-offset, es-offset); diagonal tiles compact
                tiles = [(kt, 512, 0, kt * 512) for kt in range(4 * qr)]
                tiles += [(4 * qr, 512, 0, base),
                          (4 * qr + 1, 384, 128, base + 512),
                          (4 * qr + 2, 256, 256, base + 896),
                          (4 * qr + 3, 128, 384, base + 1152)]
                groups = [tiles[i : i + 2] for i in range(0, nk, 2)]
                es = es_pool.tile([128, 8192], BF16, name="es")
                ops = opsum.tile([128, 512], PF32, name="ot")
                prev_pv = None
                for gi, g in enumerate(groups):
                    gw = sum(t[1] for t in g)
                    ge = g[0][3]
                    gps = spair.tile([128, 1024], PF32, name="sp")
                    off = 0
                    for kt, w, qo, eo in g:
                        nc.tensor.matmul(
                            gps[:, off : off + w],
                            qk_res[4 + h][:, kt * 128 : (kt + 1) * 128],
                            qk_res[h][:, q0 + qo : q0 + 512],
                            start=True, stop=True,
                        )
                        off += w
                    if gi == 1:
                        # previous q-chunk's normalization lands here: by
                        # now its denominator tree has drained and the PE
                        # has ~3us of queued QK work to cover the wait.
                        flush_norm()
                    nc.scalar.activation(es[:, ge : ge + gw], gps[:, 0:gw],
                                         AF.Exp)
                    for kt, w, qo, eo in g:
                        if kt >= 4 * qr:  # causal triangle of this k-tile
                            # DVE, not gpsimd: this sits on the critical
                            # exp->mask->PV path and Pool's dispatch+queue
                            # latency stalls the PV matmuls (+11us total).
                            nc.vector.tensor_mul(es[:, eo : eo + 128],
                                                 es[:, eo : eo + 128],
                                                 mask_sb[:])
                    if prev_pv is not None:
                        prev_pv()
                    def _pv(g=g):
                        for kt, w, qo, eo in g:
                            nc.tensor.matmul(
                                ops[:, qo:512],
                                v_res2[:, kt, h * 128 : (h + 1) * 128],
                                es[:, eo : eo + w],
                                start=(kt == 0), stop=(kt == nk - 1),
                            )
                    prev_pv = _pv
                    if gi == len(groups) - 3:
                        # denominator tree over the full-width (off-diag)
                        # slots, emitted as soon as their exps are queued;
                        # wide bf16 DVE ops amortize dispatch.
                        dn = den_pool.tile([128, 512], BF16, name="den")
                        if qr == 3:
                            t1 = tr_pool.tile([128, 2048], BF16, name="tr")
                            nc.vector.tensor_add(t1[:], es[:, 0:2048],
                                                 es[:, 4096:6144])
                            t2 = tr_pool.tile([128, 2048], BF16, name="tr")
                            nc.vector.tensor_add(t2[:], t1[:],
                                                 es[:, 2048:4096])
                            t3 = tr_pool.tile([128, 1024], BF16, name="tr2")
                            nc.vector.tensor_add(t3[:], t2[:, 0:1024],
                                                 t2[:, 1024:2048])
                            nc.vector.tensor_add(dn[:], t3[:, 0:512],
                                                 t3[:, 512:1024])
                        elif qr == 2:
                            t1 = tr_pool.tile([128, 2048], BF16, name="tr")
                            nc.vector.tensor_add(t1[:], es[:, 0:2048],
                                                 es[:, 2048:4096])
                            t3 = tr_pool.tile([128, 1024], BF16, name="tr2")
                            nc.vector.tensor_add(t3[:], t1[:, 0:1024],
                                                 t1[:, 1024:2048])
                            nc.vector.tensor_add(dn[:], t3[:, 0:512],
                                                 t3[:, 512:1024])
                        elif qr == 1:
                            t3 = tr_pool.tile([128, 1024], BF16, name="tr2")
                            nc.vector.tensor_add(t3[:], es[:, 0:1024],
                                                 es[:, 1024:2048])
                            nc.vector.tensor_add(dn[:], t3[:, 0:512],
                                                 t3[:, 512:1024])
                prev_pv()
                if qr > 0:
                    nc.vector.tensor_add(dn[:], dn[:], es[:, base : base + 512])
                    nc.vector.tensor_add(dn[:, 128:512], dn[:, 128:512],
                                         es[:, base + 512 : base + 896])
                    nc.vector.tensor_add(dn[:, 256:512], dn[:, 256:512],
                                         es[:, base + 896 : base + 1152])
                    nc.vector.tensor_add(dn[:, 384:512], dn[:, 384:512],
                                         es[:, base + 1152 : base + 1280])
                    den_ap = dn[:]
                else:
                    # no off-diagonal part: accumulate the three short diag
                    # tiles into the first slot (gpsimd — DVE is loaded).
                    nc.gpsimd.tensor_add(es[:, 128:512], es[:, 128:512],
                                         es[:, 512:896])
                    nc.gpsimd.tensor_add(es[:, 256:512], es[:, 256:512],
                                         es[:, 896:1152])
                    nc.gpsimd.tensor_add(es[:, 384:512], es[:, 384:512],
                                         es[:, 1152:1280])
                    den_ap = es[:, 0:512]
                pending_norm.append(mk_norm(h, qr, den_ap, ops))

            def attn_head_end(h):
                flush_norm()
                nc.gpsimd.collective_compute(
                    "AllToAll",
                    mybir.AluOpType.bypass,
                    replica_groups=[list(range(NC))],
                    ins=[a2a_in[h].opt()],
                    outs=[a2a_out[h].opt()],
                )
                of = of_pool.tile([128, 8, SQ], BF16, name="of")
                nc.sync.dma_start(of[:], a2a_out[h].rearrange("i p q -> p i q"))
                of_tiles[h] = of

            def oproj_wo_load(h, od, tail=False):
                wos = []
                for ih in range(2):
                    wo_sb = wo_pool.tile([128, 4, 512], BF16, name="wo")
                    # in the tail split the wo stream across both HWDGE
                    # queues (sync alone starves it); while interleaved
                    # with attention keep the scalar queue clear — its
                    # engine is saturated with the exp ACTIVATEs.
                    eng = nc.sync if (ih == 0 or not tail) else nc.scalar
                    eng.dma_start(
                        wo_sb[:],
                        WOT.ap()[h, od, ih * 4 : (ih + 1) * 4].rearrange(
                            "i p c -> p i c"
                        ),
                    )
                    wos.append(wo_sb)
                return wos

            def oproj_od(h, od, wos=None):
                if wos is None:
                    wos = oproj_wo_load(h, od)
                osl = slice(od * 512, (od + 1) * 512)
                for q2 in range(2):
                    q2sl = slice(q2 * 128, (q2 + 1) * 128)
                    ps = aux_ps.tile([128, 512], PF32, name="aux")
                    for i in range(8):
                        nc.tensor.matmul(
                            ps[:], of_tiles[h][:, i, q2sl], wos[i // 4][:, i % 4, :],
                            start=(i == 0), stop=(i == 7),
                        )
                    if h == 0:
                        nc.vector.tensor_add(yac[q2][:, osl], ps[:],
                                             bias_sb[:, osl])
                    elif h < 3:
                        nc.vector.tensor_add(yac[q2][:, osl], yac[q2][:, osl],
                                             ps[:])
                    else:
                        y_sb = yev_pool.tile([128, 512], F32, name="y")
                        nc.vector.tensor_add(y_sb[:], ps[:], yac[q2][:, osl])
                        # Y stores ride the software-DGE queue: on a HWDGE
                        # queue they head-of-line block the od(3) wo loads
                        # (they wait on data that waits on the last a2a).
                        nc.gpsimd.dma_start(Y.ap()[q2sl, osl], y_sb[:])

            # qr largest-first.  With v4's ~21us heads, a2a(0) is only
            # ready mid-head-3, so just oproj(0) interleaves there; the
            # tail runs oproj(1..3) with oproj(1)+oproj(2) (~68us of
            # a2a(3)-independent PE work) covering the last collective.
            # The 4 collectives serialize on the one CC stream (~38us
            # each) but stay just ahead of their consumers.
            for h in range(HPC):
                for i, qr in enumerate((3, 2, 1, 0)):
                    attn_qr(h, qr)
                    if h == 3:
                        oproj_od(0, 2 * i)
                        oproj_od(0, 2 * i + 1)
                attn_head_end(h)
            # tail: wo slabs prefetched two od-slices ahead (1MB per slice
            # against a 4.3us slice pace saturates a single queue).
            tail_list = [(hh, od) for hh in (1, 2, 3) for od in range(8)]
            wo_pending = {}

            def ensure_wo(i):
                if i < len(tail_list) and i not in wo_pending:
                    wo_pending[i] = oproj_wo_load(*tail_list[i], tail=True)

            ensure_wo(0)
            ensure_wo(1)
            ensure_wo(2)
            for i, (hh, od) in enumerate(tail_list):
                ensure_wo(i + 3)
                oproj_od(hh, od, wos=wo_pending.pop(i))

    nc.compile()
    return nc


def _prep_inputs(X, Wq, Wk, Wv, Wo, bo, cos, sin):
    import ml_dtypes
    BF = ml_dtypes.bfloat16

    X = np.asarray(X, dtype=np.float32)
    cos = np.asarray(cos, dtype=np.float32)
    sin = np.asarray(sin, dtype=np.float32)

    XTn = np.ascontiguousarray(X.reshape(S, D).T).astype(BF)   # [D, S]
    cosT = np.ascontiguousarray(cos.T)                         # [128, S]
    sinT = np.ascontiguousarray(sin.T).copy()
    # fold the rotate-half sign into sin: rope = x*cos + swap(x)*sin'
    # where swap is a pure partition exchange and sin'[:64] = -sin[:64].
    sinT[0:64, :] *= -1.0
    # 1/sqrt(HD) is folded into the Wq slabs (below), so Q and K share one
    # cos/sin pair.
    scale = np.float32(1.0 / np.sqrt(HD))

    # triangular mask for the first 128 columns of each diagonal k-tile's
    # restricted range: valid iff k <= q'
    MASK1 = (np.arange(128)[:, None] <= np.arange(128)[None, :]) \
        .astype(np.float32).astype(BF)
    ONESM = np.ones((128, 128), np.float32).astype(BF)

    # [h, od, i, p, c] with global k-tile = 4*i + h (source core i, head h)
    WoT8 = np.ascontiguousarray(
        np.asarray(Wo, np.float32)
        .reshape(8, 512, 8, HPC, 128)
        .transpose(3, 0, 2, 4, 1)
    ).astype(BF)
    BOB = np.broadcast_to(np.asarray(bo, np.float32)[None, :], (128, D)).copy()

    shared = dict(
        XT=XTn, COS=cosT, SIN=sinT,
        MASK1=MASK1, ONESM=ONESM, WOT=WoT8, BOB=BOB,
    )
    in_maps = []
    for c in range(NC):
        lo, hi = c * 512, (c + 1) * 512
        Wcat = np.concatenate(
            [np.asarray(Wq, np.float32)[lo:hi] * scale,
             np.asarray(Wk, np.float32)[lo:hi]], axis=0
        )  # [1024, D]
        W4 = np.ascontiguousarray(
            Wcat.reshape(8, 128, KT, 128).transpose(0, 3, 2, 1)
        ).astype(BF)  # [m, p(k), k-tile, c(out)]
        # Wv^T [k-part, k-tile, out-col]: V is projected pre-transposed
        WVT = np.ascontiguousarray(
            np.asarray(Wv, np.float32)[lo:hi].T.reshape(KT, 128, 512)
            .transpose(1, 0, 2)
        ).astype(BF)
        in_maps.append({**shared, "W4": W4, "WVT": WVT})
    return in_maps


def kernel(X, Wq, Wk, Wv, Wo, bo, cos, sin, _trace=False):
    from concourse.bass_utils import run_bass_kernel_spmd

    if "nc" not in _cache:
        _cache["nc"] = _build_program()
    nc = _cache["nc"]

    in_maps = _prep_inputs(X, Wq, Wk, Wv, Wo, bo, cos, sin)
    res = run_bass_kernel_spmd(nc, in_maps, list(range(NC)), trace=_trace)
    _cache["last_result"] = res
    Yfull = np.concatenate([res.results[c]["Y"] for c in range(NC)], axis=0)
    return Yfull.reshape(B, S, D).astype(np.float32)



# revision 50
# speedup vs baseline: 1.0077x; 1.0077x over previous
"""Tensor-parallel causal multi-head attention (RoPE) for 8 Trainium2 NeuronCores.

Problem: B=1, S=2048, D=4096, H=32 heads, head_dim=128, causal, RoPE,
out-projection with bias.  Reference: y = softmax(mask(QK^T/sqrt(hd))) V Wo^T + bo
with Q/K/V = X @ W{q,k,v}^T (nn.Linear convention) and RoPE applied to Q, K.

Sharding: tensor-parallel across heads (4 heads / core) for QKV + attention;
AllToAll re-shards to sequence (256 rows / core) for the out-projection.
Each core returns its 256-row slice of the final output; host concatenates.

v5 vs v2 (783us -> ~696us on HW; PE is GPIO-power-throttled to ~1.95GHz
for ~85% of every run, so the N=512 bf16 matmul floor is 263ns):
- scores/exp/PV restricted to the causal region at k-tile granularity
  (diagonal 512-block tiles shrink to 512/384/256/128 columns, exactly
  the per-k-tile causal limit).
- exp fused over two k-tiles per ACTIVATE ([128,1024] fp32 PSUM pair):
  the 352-cycle fixed cost was 40% of each [128,512] exp.
- per-k-tile PE colsum matmuls (~42us of PE streaming) replaced by a
  bf16 tree-sum on the DVE plus ONE all-ones [128,128] matmul per
  q-chunk that does the cross-partition reduce AND the partition
  broadcast in one pass; normalization is deferred into the next
  q-chunk's instruction stream so the PE never waits on the tree.
- V projection computed pre-transposed (stationary = X k-tile, moving =
  resident Wv^T): the 64 PE transposes of v2 vanish.
- resident cos/sin (loaded first, via SWDGE) with the 1/sqrt(hd) scale
  folded into Wq host-side; RoPE rot DMAs ride the otherwise-idle
  scalar HWDGE queue (behind bulk prefetch they stalled PSUM recycling
  ~30us; on gpsimd they deadlock behind the launch-skew barrier).
- out-projection weight slabs prefetched two od-slices ahead in the
  tail; oproj(1)+oproj(2) (~68us of PE work) emitted after the last
  AllToAll's trigger covers its ~15-45us latency.
Measured end-to-end rel_l2 ~5.9e-3 vs the 2e-2 gate.
"""

import sys
import numpy as np

for _p in ("/opt/trn_rl_repo",):
    if _p not in sys.path:
        sys.path.insert(0, _p)

B, S, D, H = 1, 2048, 4096, 32
HD = 128          # head dim
NC = 8            # cores
HPC = H // NC     # heads per core = 4
MPC = 3 * HPC     # projection m-tiles per core (Q0..3, K0..3, V0..3) = 12
SQ = S // NC      # seq rows per core after AllToAll = 256
KT = D // 128     # contraction tiles = 32

_cache = {}


def _build_program():
    import concourse.bass as bass
    import concourse.mybir as mybir
    import concourse.tile as tile
    from concourse import bacc
    from contextlib import ExitStack

    F32 = mybir.dt.float32
    F32R = mybir.dt.float32r
    BF16 = mybir.dt.bfloat16
    PF32 = mybir.dt.float32
    AF = mybir.ActivationFunctionType

    nc = bacc.Bacc("TRN2", target_bir_lowering=False, debug=False, num_devices=NC)

    XT = nc.dram_tensor("XT", [D, S], BF16, kind="ExternalInput")
    W4 = nc.dram_tensor("W4", [8, 128, KT, 128], BF16, kind="ExternalInput")
    WVT = nc.dram_tensor("WVT", [128, KT, 512], BF16, kind="ExternalInput")
    COS = nc.dram_tensor("COS", [128, S], F32, kind="ExternalInput")
    SIN = nc.dram_tensor("SIN", [128, S], F32, kind="ExternalInput")
    MASK1 = nc.dram_tensor("MASK1", [128, 128], BF16, kind="ExternalInput")
    ONESM = nc.dram_tensor("ONESM", [128, 128], BF16, kind="ExternalInput")
    WOT = nc.dram_tensor("WOT", [HPC, 8, 8, 128, 512], BF16, kind="ExternalInput")
    BOB = nc.dram_tensor("BOB", [128, D], F32, kind="ExternalInput")
    Y = nc.dram_tensor("Y", [SQ, D], F32, kind="ExternalOutput")

    with tile.TileContext(nc) as tc, ExitStack() as top:
        dram = top.enter_context(tc.tile_pool(name="dram", bufs=1, space="DRAM"))
        a2a_in = [dram.tile([NC, HD, SQ], BF16, name=f"a2ai{h}") for h in range(HPC)]
        a2a_out = [dram.tile([NC, HD, SQ], BF16, name=f"a2ao{h}") for h in range(HPC)]
        sync_in = dram.tile([NC, 1, 8], BF16, name="syncin")
        sync_out = dram.tile([NC, 1, 8], BF16, name="syncout")

        # constants via the software-DGE queue (gpsimd): keeps both HWDGE
        # queues clear for the startup X/W stream.  cos/sin go FIRST -- the
        # very first RoPE evacuate (~18us in) consumes them; in v2 they sat
        # behind 3MB of X pieces on the scalar HWDGE queue and the first
        # evacuate chain stalled the PE for 19us.
        # Persistent Q^T/K^T ([hd, s], RoPE'd) and V ([s%128, s//128, 4*hd]:
        # the V projection is computed directly transposed — stationary = X
        # k-tile, moving = Wv^T — so the 64 PE transposes of v2-v4 vanish).
        res = top.enter_context(tc.tile_pool(name="res", bufs=1))
        qk_res = [res.tile([128, S], BF16, name=f"qk{m}") for m in range(8)]
        v_res2 = res.tile([128, S // 128, 512], BF16, name="v2")

        XT_t = XT.ap().rearrange("(k p) s -> p k s", p=128)

        # ---- Phase 1: QKV projections + RoPE (Q,K) + transpose (V) ----
        # s-chunk outer (4 x 512 cols), m-tile inner; the full K=4096
        # contraction accumulates in one PSUM bank (32 matmuls).  X streams
        # once (double-buffered 4MB chunks), W streams once per chunk.
        with nc.named_scope("proj"), ExitStack() as ph:
            xt_pool = ph.enter_context(tc.tile_pool(name="xt", bufs=2))
            w_pool = ph.enter_context(tc.tile_pool(name="w", bufs=3))
            cs_pool = ph.enter_context(tc.tile_pool(name="cs", bufs=1))
            rtmp = ph.enter_context(tc.tile_pool(name="rtmp", bufs=2))
            # resident cos/sin, scoped to the projection phase (freed for
            # the attention pools); software-DGE so they land before the
            # first evacuate without fighting the X/W startup stream.
            cos_sb = cs_pool.tile([128, S], F32, name="cos")
            sin_sb = cs_pool.tile([128, S], F32, name="sin")

            # staged per s-chunk: only chunk 0's slice competes with the
            # startup X/W stream (the full 2MB stole ~6us of the HBM
            # bandwidth that the first chain is starved for).  Later
            # slices are emitted ahead of the gpsimd barrier dwell.
            def load_cs(a):
                sl = slice(a * 512, (a + 1) * 512)
                nc.gpsimd.dma_start(cos_sb[:, sl], COS.ap()[:, sl])
                nc.gpsimd.dma_start(sin_sb[:, sl], SIN.ap()[:, sl])

            load_cs(0)
            full_pool = ph.enter_context(tc.tile_pool(name="full", bufs=2))
            ppsum = ph.enter_context(tc.tile_pool(name="ppsum", bufs=4, space="PSUM"))
            # Wv^T stays resident (4MB): every s-tile's chain contracts all
            # of it, so streaming it per chunk would cost 16MB.
            wvt_sb = cs_pool.tile([128, KT, 512], BF16, name="wvt")

            def evacuate_qk(m, a, ps):
                s0 = a * 512
                sl = slice(s0, s0 + 512)
                # 1/sqrt(hd) is folded into the Wq slabs host-side, so Q
                # and K share one resident cos/sin pair.
                full_sb = full_pool.tile([128, 512], F32R, name="full")
                # Chunk 3's PSUM evacuation copies go to ACT (idle until
                # the first attention exp): the DVE's copy backlog at the
                # proj->attn boundary was stalling attention heads 0-1.
                # (ACT for ALL chunks measured 5us slower — it perturbs the
                # steady-state evacuate path.)
                if a == 3:
                    nc.scalar.copy(full_sb[:], ps[:])
                else:
                    nc.vector.tensor_copy(full_sb[:], ps[:])
                # rotate-half as a partition swap via SBUF->SBUF DMA
                # (sign is folded into SIN host-side) — keeps the
                # rotation off the PE (was a 548ns fp32 matmul each).
                # Both halves ride the scalar HWDGE queue, which carries
                # no bulk traffic after startup: behind 512KB prefetch
                # pieces the whole evacuate chain (and PSUM recycling)
                # stalls ~30us; on gpsimd it deadlocks ~40us behind the
                # launch-skew barrier collective.
                rot_sb = rtmp.tile([128, 512], F32R, name="rot")
                nc.scalar.dma_start(rot_sb[0:64, :], full_sb[64:128, :])
                nc.scalar.dma_start(rot_sb[64:128, :], full_sb[0:64, :])
                # last chunk's LATE slots (consumed only by attention heads
                # 2-3, ~60us later) run their RoPE arithmetic on gpsimd:
                # their DVE tail otherwise overlaps attention heads 0-1 and
                # the queued-behind masks/tree stall PV (~15us of PE
                # bubbles).  Head 0/1 inputs (Q0/K0/Q1/K1) stay on the DVE
                # so the attention start is not delayed by Pool's 2x-slower
                # elementwise path.
                # (Q0/K0/Q1/K1 stay on DVE — heads 0-1 need them promptly;
                # pushing Q1/K1 to Pool as well measured 5us slower.)
                eng = nc.gpsimd if (a == 3 and m in (2, 6, 3, 7)) else nc.vector
                c1 = rtmp.tile([128, 512], F32, name="c1")
                eng.tensor_mul(c1[:], full_sb[:], cos_sb[:, sl])
                r1 = rtmp.tile([128, 512], F32, name="r1")
                eng.tensor_mul(r1[:], rot_sb[:], sin_sb[:, sl])
                eng.tensor_add(qk_res[m][:, sl], c1[:], r1[:])

            def load_w(m):
                w_sb = w_pool.tile([128, KT, 128], BF16, name="w")
                nc.sync.dma_start(w_sb[:], W4.ap()[m])
                return w_sb

            # Startup interleave: first W slab goes out first (the first
            # matmul chain needs it plus only the first X piece), X pieces
            # pace the first chain, later W slabs slot between pieces.
            kq = KT // 8
            xt_cur = xt_pool.tile([128, KT, 512], BF16, name="xt")

            def xt0_piece(s, eng):
                eng.dma_start(xt_cur[:, s * kq : (s + 1) * kq, :],
                              XT_t[:, s * kq : (s + 1) * kq, 0:512])

            # startup: chunk-0 pieces split across both HWDGE queues so the
            # 4MB load lands in ~half the serial time; W slabs interleave
            # on the sync queue.  Slab 0 goes in quarters so the first
            # matmul only waits on a 256KB load.
            # strict alternation: even pieces on scalar, odd pieces on
            # sync woven between the w0 quarters, W slabs for slots 1-2
            # after the last piece — both queues then deliver chain 0's
            # inputs in consumption order.
            w0_sb = w_pool.tile([128, KT, 128], BF16, name="w")
            nc.sync.dma_start(w0_sb[:, 0:8, :], W4.ap()[0, :, 0:8, :])
            xt0_piece(0, nc.scalar)
            xt0_piece(1, nc.sync)
            xt0_piece(2, nc.scalar)
            nc.sync.dma_start(w0_sb[:, 8:16, :], W4.ap()[0, :, 8:16, :])
            xt0_piece(3, nc.sync)
            xt0_piece(4, nc.scalar)
            nc.sync.dma_start(w0_sb[:, 16:24, :], W4.ap()[0, :, 16:24, :])
            w_pre = [w0_sb]
            xt0_piece(5, nc.sync)
            xt0_piece(6, nc.scalar)
            nc.sync.dma_start(w0_sb[:, 24:32, :], W4.ap()[0, :, 24:32, :])
            xt0_piece(7, nc.sync)
            w_pre.append(load_w(4))
            w_pre.append(load_w(1))
            # slot 3's K1 slab must precede the 4MB Wv^T stream on sync or
            # the PE stalls 13us at ~55us waiting for it.
            w_pre.append(load_w(5))
            # resident Wv^T streams in behind the chunk-0 startup set; the
            # first V s-tile runs at slot 4 (~35us in).  Pieces interleave
            # with the in-loop W slab loads (below) so slot 2/3's K slabs
            # aren't queued behind all 4MB.
            def load_wvt(p):
                nc.sync.dma_start(wvt_sb[:, p * 8 : (p + 1) * 8, :],
                                  WVT.ap()[:, p * 8 : (p + 1) * 8, :])

            load_wvt(0)
            # chunk 0 defers its V s-tiles until Wv^T has landed.
            slots0 = [("qk", 0), ("qk", 4), ("qk", 1), ("qk", 5), ("v", 0),
                      ("v", 1), ("qk", 2), ("qk", 6), ("v", 2), ("qk", 3),
                      ("qk", 7), ("v", 3)]
            slotsN = [("qk", 0), ("qk", 4), ("v", 0), ("qk", 1), ("qk", 5),
                      ("v", 1), ("qk", 2), ("qk", 6), ("v", 2), ("qk", 3),
                      ("qk", 7), ("v", 3)]
            xt_next = None
            for a in range(4):
                for mi, (kind, m) in enumerate(slots0 if a == 0 else slotsN):
                    if a < 3:
                        # spread next-chunk prefetch: one 512KB piece per m,
                        # on the sync queue behind the W slabs (which run
                        # 2-3 slots ahead) — the scalar queue stays clear
                        # for the latency-critical RoPE rot DMAs.
                        if mi == 2:
                            xt_next = xt_pool.tile([128, KT, 512], BF16,
                                                   name="xt")
                        if 2 <= mi < 10:
                            s = mi - 2
                            nc.sync.dma_start(
                                xt_next[:, s * kq : (s + 1) * kq, :],
                                XT_t[:, s * kq : (s + 1) * kq,
                                     (a + 1) * 512 : (a + 2) * 512],
                            )
                    ps = ppsum.tile([128, 512], PF32, name="proj")
                    if a == 0 and mi in (0, 1, 2):
                        load_wvt(mi + 1)
                    if a < 3 and mi == 0:
                        load_cs(a + 1)
                    if kind == "qk":
                        w_sb = (w_pre[{0: 0, 4: 1, 1: 2, 5: 3}[m]]
                                if (a == 0 and mi < 4) else load_w(m))
                        for kt in range(KT):
                            nc.tensor.matmul(ps[:], w_sb[:, kt, :],
                                             xt_cur[:, kt, :],
                                             start=(kt == 0),
                                             stop=(kt == KT - 1))
                        evacuate_qk(m, a, ps)
                    else:
                        # V s-tile, computed transposed: stationary = X
                        # k-tile [k, s], moving = Wv^T [k, 4*hd]
                        t = 4 * a + m
                        jj = slice(m * 128, (m + 1) * 128)
                        for kt in range(KT):
                            nc.tensor.matmul(ps[:], xt_cur[:, kt, jj],
                                             wvt_sb[:, kt, :],
                                             start=(kt == 0),
                                             stop=(kt == KT - 1))
                        if a == 3:
                            nc.scalar.copy(v_res2[:, t, :], ps[:])
                        else:
                            nc.vector.tensor_copy(v_res2[:, t, :], ps[:])
                    if a == 0 and mi == 0:
                        # tiny collective doubling as a cross-core barrier:
                        # absorbs SPMD launch skew here, where the PE still
                        # has ~400us of queued work, so the first real
                        # AllToAll is not a skew sink.
                        zz = full_pool.tile([1, NC * 8], BF16, name="zz")
                        nc.any.memset(zz[:], 0.0)
                        nc.sync.dma_start(sync_in.rearrange("i p q -> p (i q)"),
                                          zz[:])
                        nc.gpsimd.collective_compute(
                            "AllToAll",
                            mybir.AluOpType.bypass,
                            replica_groups=[list(range(NC))],
                            ins=[sync_in.opt()],
                            outs=[sync_out.opt()],
                        )
                xt_cur = xt_next

        # ---- Phase 2+3: attention interleaved with out-projection ----
        # v4: scores/exp/PV restricted to the causal region at k-tile
        # granularity (the diagonal 512-block's tiles shrink to
        # 512/384/256/128 columns), exp fused over two k-tiles per
        # ACTIVATE (the 352-cycle fixed cost was 40% of each exp), and the
        # per-tile PE colsum matmuls (42us of streaming) replaced by a
        # bf16 tree-sum on the DVE plus ONE all-ones matmul per q-chunk
        # that does the cross-partition reduce and the partition broadcast
        # in the same pass.
        with nc.named_scope("attn"), ExitStack() as ph:
            es_pool = ph.enter_context(tc.tile_pool(name="es", bufs=2))
            tr_pool = ph.enter_context(tc.tile_pool(name="tr", bufs=2))
            den_pool = ph.enter_context(tc.tile_pool(name="den", bufs=2))
            on_pool = ph.enter_context(tc.tile_pool(name="on", bufs=4))
            of_pool = ph.enter_context(tc.tile_pool(name="of", bufs=2))
            wo_pool = ph.enter_context(tc.tile_pool(name="wo", bufs=8))
            yac_pool = ph.enter_context(tc.tile_pool(name="yac", bufs=1))
            yev_pool = ph.enter_context(tc.tile_pool(name="yev", bufs=4))
            msk_pool = ph.enter_context(tc.tile_pool(name="msk", bufs=1))
            spair = ph.enter_context(tc.tile_pool(name="spair", bufs=2, space="PSUM"))
            opsum = ph.enter_context(tc.tile_pool(name="opsum", bufs=2, space="PSUM"))
            aux_ps = ph.enter_context(tc.tile_pool(name="auxps", bufs=2, space="PSUM"))

            mask_sb = msk_pool.tile([128, 128], BF16, name="mask1")
            nc.sync.dma_start(mask_sb[:], MASK1.ap())
            onesm_sb = msk_pool.tile([128, 128], BF16, name="onesm")
            nc.sync.dma_start(onesm_sb[:], ONESM.ap())
            bias_sb = msk_pool.tile([128, D], F32, name="bias")
            nc.sync.dma_start(bias_sb[:], BOB.ap())
            yac = [yac_pool.tile([128, D], F32, name=f"yac{q2}") for q2 in range(2)]
            of_tiles = {}
            pending_norm = []

            def flush_norm():
                while pending_norm:
                    pending_norm.pop(0)()

            def mk_norm(h, qr, den_ap, ops):
                # one matmul against all-ones [128,128] = colsum over the
                # 128 k-partitions AND broadcast of the result to every
                # output partition; reciprocal+scale on DVE.
                def norm():
                    bps = aux_ps.tile([128, 512], PF32, name="aux")
                    nc.tensor.matmul(bps[:], onesm_sb[:], den_ap,
                                     start=True, stop=True)
                    rec_sb = on_pool.tile([128, 512], F32, name="rec")
                    nc.vector.reciprocal_approx_fast(out=rec_sb[:], in_=bps[:])
                    otn = on_pool.tile([128, 512], BF16, name="otn")
                    nc.vector.tensor_mul(otn[:], ops[:], rec_sb[:])
                    for half in range(2):
                        nc.sync.dma_start(
                            a2a_in[h][2 * qr + half, :, :],
                            otn[:, half * 256 : (half + 1) * 256],
                        )
                return norm

            def attn_qr(h, qr):
                nk = 4 * (qr + 1)
                q0 = qr * 512
                base = 4 * qr * 512  # es offset of the diagonal region
                # (kt, width, q-offset, es-offset); diagonal tiles compact
                tiles = [(kt, 512, 0, kt * 512) for kt in range(4 * qr)]
                tiles += [(4 * qr, 512, 0, base),
                          (4 * qr + 1, 384, 128, base + 512),
                          (4 * qr + 2, 256, 256, base + 896),
                          (4 * qr + 3, 128, 384, base + 1152)]
                groups = [tiles[i : i + 2] for i in range(0, nk, 2)]
                es = es_pool.tile([128, 8192], BF16, name="es")
                ops = opsum.tile([128, 512], PF32, name="ot")
                prev_pv = None
                for gi, g in enumerate(groups):
                    gw = sum(t[1] for t in g)
                    ge = g[0][3]
                    gps = spair.tile([128, 1024], PF32, name="sp")
                    off = 0
                    for kt, w, qo, eo in g:
                        nc.tensor.matmul(
                            gps[:, off : off + w],
                            qk_res[4 + h][:, kt * 128 : (kt + 1) * 128],
                            qk_res[h][:, q0 + qo : q0 + 512],
                            start=True, stop=True,
                        )
                        off += w
                    if gi == 1:
                        # previous q-chunk's normalization lands here: by
                        # now its denominator tree has drained and the PE
                        # has ~3us of queued QK work to cover the wait.
                        flush_norm()
                    nc.scalar.activation(es[:, ge : ge + gw], gps[:, 0:gw],
                                         AF.Exp)
                    for kt, w, qo, eo in g:
                        if kt >= 4 * qr:  # causal triangle of this k-tile
                            # DVE, not gpsimd: this sits on the critical
                            # exp->mask->PV path and Pool's dispatch+queue
                            # latency stalls the PV matmuls (+11us total).
                            nc.vector.tensor_mul(es[:, eo : eo + 128],
                                                 es[:, eo : eo + 128],
                                                 mask_sb[:])
                    if prev_pv is not None:
                        prev_pv()
                    def _pv(g=g):
                        for kt, w, qo, eo in g:
                            nc.tensor.matmul(
                                ops[:, qo:512],
                                v_res2[:, kt, h * 128 : (h + 1) * 128],
                                es[:, eo : eo + w],
                                start=(kt == 0), stop=(kt == nk - 1),
                            )
                    prev_pv = _pv
                    if gi == len(groups) - 3:
                        # denominator tree over the full-width (off-diag)
                        # slots, emitted as soon as their exps are queued;
                        # wide bf16 DVE ops amortize dispatch.
                        dn = den_pool.tile([128, 512], BF16, name="den")
                        if qr == 3:
                            t1 = tr_pool.tile([128, 2048], BF16, name="tr")
                            nc.vector.tensor_add(t1[:], es[:, 0:2048],
                                                 es[:, 4096:6144])
                            t2 = tr_pool.tile([128, 2048], BF16, name="tr")
                            nc.vector.tensor_add(t2[:], t1[:],
                                                 es[:, 2048:4096])
                            t3 = tr_pool.tile([128, 1024], BF16, name="tr2")
                            nc.vector.tensor_add(t3[:], t2[:, 0:1024],
                                                 t2[:, 1024:2048])
                            nc.vector.tensor_add(dn[:], t3[:, 0:512],
                                                 t3[:, 512:1024])
                        elif qr == 2:
                            t1 = tr_pool.tile([128, 2048], BF16, name="tr")
                            nc.vector.tensor_add(t1[:], es[:, 0:2048],
                                                 es[:, 2048:4096])
                            t3 = tr_pool.tile([128, 1024], BF16, name="tr2")
                            nc.vector.tensor_add(t3[:], t1[:, 0:1024],
                                                 t1[:, 1024:2048])
                            nc.vector.tensor_add(dn[:], t3[:, 0:512],
                                                 t3[:, 512:1024])
                        elif qr == 1:
                            t3 = tr_pool.tile([128, 1024], BF16, name="tr2")
                            nc.vector.tensor_add(t3[:], es[:, 0:1024],
                                                 es[:, 1024:2048])
                            nc.vector.tensor_add(dn[:], t3[:, 0:512],
                                                 t3[:, 512:1024])
                prev_pv()
                if qr > 0:
                    nc.vector.tensor_add(dn[:], dn[:], es[:, base : base + 512])
                    nc.vector.tensor_add(dn[:, 128:512], dn[:, 128:512],
                                         es[:, base + 512 : base + 896])
                    nc.vector.tensor_add(dn[:, 256:512], dn[:, 256:512],
                                         es[:, base + 896 : base + 1152])
                    nc.vector.tensor_add(dn[:, 384:512], dn[:, 384:512],
                                         es[:, base + 1152 : base + 1280])
                    den_ap = dn[:]
                else:
                    # no off-diagonal part: accumulate the three short diag
                    # tiles into the first slot (gpsimd — DVE is loaded).
                    nc.gpsimd.tensor_add(es[:, 128:512], es[:, 128:512],
                                         es[:, 512:896])
                    nc.gpsimd.tensor_add(es[:, 256:512], es[:, 256:512],
                                         es[:, 896:1152])
                    nc.gpsimd.tensor_add(es[:, 384:512], es[:, 384:512],
                                         es[:, 1152:1280])
                    den_ap = es[:, 0:512]
                pending_norm.append(mk_norm(h, qr, den_ap, ops))

            def attn_head_end(h):
                flush_norm()
                nc.gpsimd.collective_compute(
                    "AllToAll",
                    mybir.AluOpType.bypass,
                    replica_groups=[list(range(NC))],
                    ins=[a2a_in[h].opt()],
                    outs=[a2a_out[h].opt()],
                )
                of = of_pool.tile([128, 8, SQ], BF16, name="of")
                nc.sync.dma_start(of[:], a2a_out[h].rearrange("i p q -> p i q"))
                of_tiles[h] = of

            def oproj_wo_load(h, od, tail=False):
                wos = []
                for ih in range(2):
                    wo_sb = wo_pool.tile([128, 4, 512], BF16, name="wo")
                    # in the tail split the wo stream across both HWDGE
                    # queues (sync alone starves it); while interleaved
                    # with attention keep the scalar queue clear — its
                    # engine is saturated with the exp ACTIVATEs.
                    eng = nc.sync if (ih == 0 or not tail) else nc.scalar
                    eng.dma_start(
                        wo_sb[:],
                        WOT.ap()[h, od, ih * 4 : (ih + 1) * 4].rearrange(
                            "i p c -> p i c"
                        ),
                    )
                    wos.append(wo_sb)
                return wos

            def oproj_od(h, od, wos=None):
                if wos is None:
                    wos = oproj_wo_load(h, od)
                osl = slice(od * 512, (od + 1) * 512)
                for q2 in range(2):
                    q2sl = slice(q2 * 128, (q2 + 1) * 128)
                    ps = aux_ps.tile([128, 512], PF32, name="aux")
                    for i in range(8):
                        nc.tensor.matmul(
                            ps[:], of_tiles[h][:, i, q2sl], wos[i // 4][:, i % 4, :],
                            start=(i == 0), stop=(i == 7),
                        )
                    if h == 0:
                        nc.vector.tensor_add(yac[q2][:, osl], ps[:],
                                             bias_sb[:, osl])
                    elif h < 3:
                        nc.vector.tensor_add(yac[q2][:, osl], yac[q2][:, osl],
                                             ps[:])
                    else:
                        y_sb = yev_pool.tile([128, 512], F32, name="y")
                        nc.vector.tensor_add(y_sb[:], ps[:], yac[q2][:, osl])
                        # Y stores ride the software-DGE queue: on a HWDGE
                        # queue they head-of-line block the od(3) wo loads
                        # (they wait on data that waits on the last a2a).
                        nc.gpsimd.dma_start(Y.ap()[q2sl, osl], y_sb[:])

            # qr largest-first.  With v4's ~21us heads, a2a(0) is only
            # ready mid-head-3, so just oproj(0) interleaves there; the
            # tail runs oproj(1..3) with oproj(1)+oproj(2) (~68us of
            # a2a(3)-independent PE work) covering the last collective.
            # The 4 collectives serialize on the one CC stream (~38us
            # each) but stay just ahead of their consumers.
            for h in range(HPC):
                for i, qr in enumerate((3, 2, 1, 0)):
                    attn_qr(h, qr)
                    if h == 3:
                        oproj_od(0, 2 * i)
                        oproj_od(0, 2 * i + 1)
                attn_head_end(h)
            # tail: wo slabs prefetched two od-slices ahead (1MB per slice
            # against a 4.3us slice pace saturates a single queue).
            tail_list = [(hh, od) for hh in (1, 2, 3) for od in range(8)]
            wo_pending = {}

            def ensure_wo(i):
                if i < len(tail_list) and i not in wo_pending:
                    wo_pending[i] = oproj_wo_load(*tail_list[i], tail=True)

            ensure_wo(0)
            ensure_wo(1)
            ensure_wo(2)
            for i, (hh, od) in enumerate(tail_list):
                ensure_wo(i + 3)
                oproj_od(hh, od, wos=wo_pending.pop(i))

    nc.compile()
    return nc


def _prep_inputs(X, Wq, Wk, Wv, Wo, bo, cos, sin):
    import ml_dtypes
    BF = ml_dtypes.bfloat16

    X = np.asarray(X, dtype=np.float32)
    cos = np.asarray(cos, dtype=np.float32)
    sin = np.asarray(sin, dtype=np.float32)

    XTn = np.ascontiguousarray(X.reshape(S, D).T).astype(BF)   # [D, S]
    cosT = np.ascontiguousarray(cos.T)                         # [128, S]
    sinT = np.ascontiguousarray(sin.T).copy()
    # fold the rotate-half sign into sin: rope = x*cos + swap(x)*sin'
    # where swap is a pure partition exchange and sin'[:64] = -sin[:64].
    sinT[0:64, :] *= -1.0
    # 1/sqrt(HD) is folded into the Wq slabs (below), so Q and K share one
    # cos/sin pair.
    scale = np.float32(1.0 / np.sqrt(HD))

    # triangular mask for the first 128 columns of each diagonal k-tile's
    # restricted range: valid iff k <= q'
    MASK1 = (np.arange(128)[:, None] <= np.arange(128)[None, :]) \
        .astype(np.float32).astype(BF)
    ONESM = np.ones((128, 128), np.float32).astype(BF)

    # [h, od, i, p, c] with global k-tile = 4*i + h (source core i, head h)
    WoT8 = np.ascontiguousarray(
        np.asarray(Wo, np.float32)
        .reshape(8, 512, 8, HPC, 128)
        .transpose(3, 0, 2, 4, 1)
    ).astype(BF)
    BOB = np.broadcast_to(np.asarray(bo, np.float32)[None, :], (128, D)).copy()

    shared = dict(
        XT=XTn, COS=cosT, SIN=sinT,
        MASK1=MASK1, ONESM=ONESM, WOT=WoT8, BOB=BOB,
    )
    in_maps = []
    for c in range(NC):
        lo, hi = c * 512, (c + 1) * 512
        Wcat = np.concatenate(
            [np.asarray(Wq, np.float32)[lo:hi] * scale,
             np.asarray(Wk, np.float32)[lo:hi]], axis=0
        )  # [1024, D]
        W4 = np.ascontiguousarray(
            Wcat.reshape(8, 128, KT, 128).transpose(0, 3, 2, 1)
        ).astype(BF)  # [m, p(k), k-tile, c(out)]
        # Wv^T [k-part, k-tile, out-col]: V is projected pre-transposed
        WVT = np.ascontiguousarray(
            np.asarray(Wv, np.float32)[lo:hi].T.reshape(KT, 128, 512)
            .transpose(1, 0, 2)
        ).astype(BF)
        in_maps.append({**shared, "W4": W4, "WVT": WVT})
    return in_maps


def kernel(X, Wq, Wk, Wv, Wo, bo, cos, sin, _trace=False):
    from concourse.bass_utils import run_bass_kernel_spmd

    if "nc" not in _cache:
        _cache["nc"] = _build_program()
    nc = _cache["nc"]

    in_maps = _prep_inputs(X, Wq, Wk, Wv, Wo, bo, cos, sin)
    res = run_bass_kernel_spmd(nc, in_maps, list(range(NC)), trace=_trace)
    _cache["last_result"] = res
    Yfull = np.concatenate([res.results[c]["Y"] for c in range(NC)], axis=0)
    return Yfull.reshape(B, S, D).astype(np.float32)

